# revision 23
# baseline (speedup 1.0000x reference)
"""AdaptiveImageTokenPruner Trainium2 kernel (8 NeuronCores, data parallel).

Key facts used (verified against the reference):
- sigmoid, +b2, and per-sample min-max normalization are monotonic -> the
  top-k selection depends only on the pre-sigmoid logits s = W2 @ gelu(W1x+b1).
- the "adaptive tree" in the reference always splits to max depth -> fixed
  32 segments of 18 tokens per sample, top-9 per segment, indices ascending.
- per-segment top-9 via DVE max8/max_index/match_replace; token gather via
  indirect DMA from HBM.

Scorer matmul precision: PE is natively fp22 (11-bit mantissa inputs, exact
products, ~f32 accumulate). Plain float32 matmuls run at 4 cycles/row (exact);
float32r runs at 1 cycle/row but truncates inputs to 11 mantissa bits.
A hi/lo split (x = x_hi + x_lo, both fp22-exact) recovers near-f32 accuracy
in 2-3 passes at full rate. mm1_mode selects: "f32" | "f32r_x2" | "f32r_x3".
"""

import numpy as np

B_FULL, N_TOK, H_DIM = 256, 576, 1024
K_DIM = 512
SEG_LEN, QUOTA = 18, 9
N_CORES = 8
B_CORE = B_FULL // N_CORES          # 32 samples per core
BLK = 512                            # tokens per scoring block

_COMPILED = {}


def _build(nc, n_samples, mm1_mode="f32", mm2_mode="dve", reps=1, ablate=""):
    import concourse.mybir as mybir
    from concourse.tile import TileContext
    from concourse.masks import make_identity
    import concourse.bass as bass
    import concourse.bass_isa as bass_isa

    f32 = mybir.dt.float32
    f32r = mybir.dt.float32r
    f16 = mybir.dt.float16
    u32 = mybir.dt.uint32
    i32 = mybir.dt.int32
    AF = mybir.ActivationFunctionType
    Alu = mybir.AluOpType

    tokens = n_samples * N_TOK
    assert tokens % BLK == 0
    n_blocks = tokens // BLK
    n_segs = n_samples * 32
    assert n_segs % 128 == 0
    n_groups = n_segs // 128
    seg_per_group = 128

    x_ext = nc.declare_dram_parameter(
        "image_features", [n_samples, N_TOK, H_DIM], f32, isOutput=False)
    w1_ext = nc.declare_dram_parameter("W1", [K_DIM, H_DIM], f32, isOutput=False)
    b1_ext = nc.declare_dram_parameter("b1", [K_DIM], f32, isOutput=False)
    w2_ext = nc.declare_dram_parameter("W2", [1, K_DIM], f32, isOutput=False)
    out_ext = nc.declare_dram_parameter(
        "out", [n_samples, QUOTA * 32, H_DIM], f32, isOutput=True)

    x_flat = x_ext.ap().rearrange("b n h -> (b n) h")            # [tokens, H]
    out_sj = out_ext.ap().rearrange("b (sg j) h -> (b sg) j h", j=QUOTA)

    n_hc = H_DIM // 128   # 8 h chunks
    n_kc = K_DIM // 128   # 4 k chunks

    with TileContext(nc) as tc:
        with (
            tc.tile_pool(name="wprep", bufs=1) as wprep,
            tc.tile_pool(name="weights", bufs=1) as wpool,
            tc.tile_pool(name="xnat", bufs=7) as p_xnat,
            tc.tile_pool(name="xt", bufs=14) as p_xt,
            tc.tile_pool(name="xtlo", bufs=14) as p_xtlo,
            tc.tile_pool(name="gt", bufs=8) as p_gt,
            tc.tile_pool(name="ps_xt", bufs=5, space="PSUM") as ps_xt,
            tc.tile_pool(name="ps_u", bufs=3, space="PSUM") as ps_u,
            tc.tile_pool(name="sseg", bufs=3) as p_sseg,
            tc.tile_pool(name="topk", bufs=3) as p_topk,
            tc.tile_pool(name="got", bufs=4) as p_got,
        ):
            # ---- one-time weight prep ----
            ident = wprep.tile([128, 128], f32, tag="ident")
            make_identity(nc, ident)

            split = mm1_mode != "f32"
            sdt = f16 if mm1_mode.startswith("fp16") else f32r
            # W1T[hc] : [128h, 512k]  (hi/lo fp32r variants when splitting;
            # fp32r-typed writes round to fp22, exactly what the PE consumes)
            w1nat = [p_xnat.tile([128, H_DIM], f32, tag="xn", name=f"w1n{c}") for c in range(n_kc)]
            for c in range(n_kc):
                nc.sync.dma_start(out=w1nat[c][:], in_=w1_ext[c * 128:(c + 1) * 128, :])
            if split:
                w1t_hi = [wpool.tile([128, K_DIM], sdt, tag=f"w1th{h}", name=f"w1th{h}") for h in range(n_hc)]
                w1t_lo = [wpool.tile([128, K_DIM], sdt, tag=f"w1tl{h}", name=f"w1tl{h}") for h in range(n_hc)]
            else:
                w1t = [wpool.tile([128, K_DIM], f32, tag=f"w1t{h}", name=f"w1t{h}") for h in range(n_hc)]
            for hc in range(n_hc):
                pw = ps_xt.tile([128, K_DIM], f32, tag="psxt", name=f"pw{hc}")
                for c in range(n_kc):
                    nc.tensor.transpose(
                        out=pw[:, c * 128:(c + 1) * 128],
                        in_=w1nat[c][:, hc * 128:(hc + 1) * 128],
                        identity=ident[:])
                if split:
                    nc.vector.tensor_copy(out=w1t_hi[hc][:], in_=pw[:])
                    hi_rd = w1t_hi[hc].bitcast(f32) if sdt == f32r else w1t_hi[hc]
                    nc.vector.tensor_tensor(
                        out=w1t_lo[hc][:], in0=pw[:],
                        in1=hi_rd[:], op=Alu.subtract)
                else:
                    nc.scalar.copy(out=w1t[hc][:], in_=pw[:])

            # b1 -> [128, n_kc]; W2 -> [128, n_kc] (partition = k within chunk)
            b1_sb = wpool.tile([128, n_kc], f32, tag="b1")
            w2_sb = wpool.tile([128, n_kc], f32, tag="w2")
            for c in range(n_kc):
                nc.sync.dma_start(
                    out=b1_sb[:, c:c + 1],
                    in_=b1_ext[c * 128:(c + 1) * 128].rearrange("(p o) -> p o", o=1))
                nc.sync.dma_start(
                    out=w2_sb[:, c:c + 1],
                    in_=w2_ext[0:1, c * 128:(c + 1) * 128].rearrange("o (p q) -> (o p) q", q=1))

            # seg base indices: base[p, g] = (g*128 + p) * 18
            base_i = wpool.tile([128, n_groups], i32, tag="basei")
            base_f = wpool.tile([128, n_groups], f32, tag="basef")
            nc.gpsimd.iota(base_i[:], pattern=[[seg_per_group * SEG_LEN, n_groups]],
                           base=0, channel_multiplier=SEG_LEN)
            nc.vector.tensor_copy(out=base_f[:], in_=base_i[:])

            # scores packed [128 seg-partitions, n_groups*18]:
            # s_pack[p, g*18+i] = score of token (g*128+p)*18 + i
            s_pack = wpool.tile([128, n_groups * SEG_LEN], f32, tag="spack")

            for rep in range(reps):
                # ---- topk/gather machinery: 64-segment units, interleaved with
                # scoring so gather DMA traffic spreads instead of bursting ----
                SEGS_U = 64
                assert n_segs % SEGS_U == 0
                # unit list: 64-seg units, but the last 64 segs split into 2x32
                # so the pipeline tail (only the final unit's topk+gather runs
                # after the last score) is halved.
                units = [(s, SEGS_U) for s in range(0, n_segs - SEGS_U, SEGS_U)]
                units += [(n_segs - SEGS_U, 32), (n_segs - 32, 32)]
                n_units = len(units)
                units_done = 0
                pending = []
                drain_rate = -(-n_units * QUOTA // n_blocks)

                def emit_topk(u):
                    seg0, nsg = units[u]
                    p0 = seg0 % 128
                    g = seg0 // 128
                    sseg = s_pack[p0:p0 + nsg, g * SEG_LEN:(g + 1) * SEG_LEN]
                    m8 = p_topk.tile([nsg, 8], f32, tag="m8", name=f"m8_{rep}_{u}")
                    i8 = p_topk.tile([nsg, 8], u32, tag="i8", name=f"i8_{rep}_{u}")
                    zap = p_topk.tile([nsg, SEG_LEN], f32, tag="zap", name=f"zap{rep}_{u}")
                    m8b = p_topk.tile([nsg, 8], f32, tag="m8b", name=f"m8b{rep}_{u}")
                    i8b = p_topk.tile([nsg, 8], u32, tag="i8b", name=f"i8b{rep}_{u}")
                    iv = p_topk.tile([nsg, QUOTA], f32, tag="iv", name=f"iv{rep}_{u}")
                    ivn = p_topk.tile([nsg, QUOTA], f32, tag="ivn", name=f"ivn{rep}_{u}")
                    asc = p_topk.tile([nsg, 8], f32, tag="asc", name=f"asc{rep}_{u}")
                    gidx_f = p_topk.tile([nsg, QUOTA], f32, tag="gidxf", name=f"gf{rep}_{u}")
                    gidx = p_topk.tile([nsg, QUOTA], u32, tag="gidx", name=f"gi{rep}_{u}",
                                       bufs=4)

                    nc.vector.max(out=m8[:], in_=sseg)
                    nc.vector.max_index(out=i8[:], in_max=m8[:], in_values=sseg)
                    nc.vector.match_replace(out=zap[:], in_to_replace=m8[:],
                                            in_values=sseg, imm_value=-1e30)
                    nc.vector.max(out=m8b[:], in_=zap[:])
                    nc.vector.max_index(out=i8b[:], in_max=m8b[:], in_values=zap[:])
                    nc.vector.tensor_copy(out=iv[:, 0:8], in_=i8[:])
                    nc.vector.tensor_copy(out=iv[:, 8:9], in_=i8b[:, 0:1])
                    nc.vector.tensor_scalar(ivn[:], iv[:], -1.0, scalar2=None,
                                            op0=Alu.mult)
                    nc.vector.max(out=asc[:], in_=ivn[:])
                    nc.vector.tensor_scalar(gidx_f[:, 0:8], asc[:], -1.0, scalar2=None,
                                            op0=Alu.mult)
                    nc.vector.reduce_max(out=gidx_f[:, 8:9], in_=iv[:],
                                         axis=mybir.AxisListType.X)
                    nc.vector.tensor_scalar(gidx_f[:], gidx_f[:],
                                            base_f[p0:p0 + nsg, g:g + 1],
                                            scalar2=None, op0=Alu.add)
                    nc.vector.tensor_copy(out=gidx[:], in_=gidx_f[:])
                    return [(u, gidx, j) for j in range(QUOTA)]

                def emit_gather(u, gidx, j):
                    if ablate == "nogather":
                        return
                    seg0, nsg = units[u]
                    got = p_got.tile([nsg, H_DIM], f32, tag="got", name=f"got{rep}_{u}_{j}")
                    nc.gpsimd.indirect_dma_start(
                        out=got[:], out_offset=None, in_=x_flat,
                        in_offset=bass.IndirectOffsetOnAxis(ap=gidx[:, j:j + 1], axis=0))
                    nc.sync.dma_start(
                        out=out_sj[seg0:seg0 + nsg, j, :], in_=got[:])

                # ---- scoring over token blocks ----
                for blk in range(n_blocks):
                    t0 = blk * BLK
                    xn = [p_xnat.tile([128, H_DIM], f32, tag="xn", name=f"xn{rep}_{blk}_{t}") for t in range(4)]
                    for t in range(4):
                        nc.sync.dma_start(
                            out=xn[t][:], in_=x_flat[t0 + t * 128: t0 + (t + 1) * 128, :])
                    # transpose into [128h, 512tok] (+ hi/lo split)
                    xts, xts_lo = [], []
                    for hc in range(n_hc):
                        pxt = ps_xt.tile([128, BLK], f32, tag="psxt")
                        for t in range(4):
                            nc.tensor.transpose(
                                out=pxt[:, t * 128:(t + 1) * 128],
                                in_=xn[t][:, hc * 128:(hc + 1) * 128],
                                identity=ident[:])
                        if split:
                            xt_hi = p_xt.tile([128, BLK], sdt, tag="xt")
                            xt_lo = p_xtlo.tile([128, BLK], sdt, tag="xtlo")
                            nc.scalar.copy(out=xt_hi[:], in_=pxt[:])
                            hi_rd = xt_hi.bitcast(f32) if sdt == f32r else xt_hi
                            nc.vector.tensor_tensor(
                                out=xt_lo[:], in0=pxt[:], in1=hi_rd[:],
                                op=Alu.subtract)
                            xts.append(xt_hi)
                            xts_lo.append(xt_lo)
                        else:
                            xt = p_xt.tile([128, BLK], f32, tag="xt")
                            nc.scalar.copy(out=xt[:], in_=pxt[:])
                            xts.append(xt)

                    # u^T[kc] = sum_h W1T[h, kslice].T @ xT[h, :]  (+gelu -> g^T)
                    gts = []
                    for c in range(n_kc):
                        pu = ps_u.tile([128, BLK], f32, tag="psu")
                        ks = slice(c * 128, (c + 1) * 128)
                        if mm1_mode == "f32":
                            for hc in range(n_hc):
                                nc.tensor.matmul(
                                    out=pu[:], lhsT=w1t[hc][:, ks], rhs=xts[hc][:],
                                    start=(hc == 0), stop=(hc == n_hc - 1))
                        else:
                            passes = [(w1t_hi, xts)]
                            if mm1_mode not in ("f32r_x1",):
                                passes.append((w1t_hi, xts_lo))
                            if mm1_mode in ("f32r_x3", "fp16_x3"):
                                passes.append((w1t_lo, xts))
                            n_mm = len(passes) * n_hc
                            m = 0
                            for wv, xv in passes:
                                for hc in range(n_hc):
                                    nc.tensor.matmul(
                                        out=pu[:], lhsT=wv[hc][:, ks], rhs=xv[hc][:],
                                        start=(m == 0), stop=(m == n_mm - 1))
                                    m += 1
                        gt = p_gt.tile([128, BLK], f32, tag="gt")
                        nc.scalar.activation(out=gt[:], in_=pu[:], func=AF.Gelu,
                                             bias=b1_sb[:, c:c + 1], scale=1.0)
                        gts.append(gt)

                    # s[1, blk] = sum_k w2[k] * g^T[k, :]
                    if mm2_mode == "f32":
                        psc = ps_u.tile([1, BLK], f32, tag="pss")
                        for c in range(n_kc):
                            nc.tensor.matmul(
                                out=psc[:], lhsT=w2_sb[:, c:c + 1], rhs=gts[c][:],
                                start=(c == 0), stop=(c == n_kc - 1))
                        s_blk = p_sseg.tile([1, BLK], f32, tag="sblk")
                        nc.scalar.copy(out=s_blk[:], in_=psc[:])
                    else:
                        # w2-scale per partition (DVE), then cross-partition add
                        # (gpsimd axis=C), then combine the 4 k-chunk partials.
                        parts = []
                        for c in range(n_kc):
                            gtw = p_gt.tile([128, BLK], f32, tag="gtw", name=f"gtw{rep}_{blk}_{c}")
                            nc.vector.tensor_scalar(
                                gtw[:], gts[c][:], w2_sb[:, c:c + 1], scalar2=None,
                                op0=Alu.mult)
                            ar = p_gt.tile([128, BLK], f32, tag="arout",
                                           name=f"ar{rep}_{blk}_{c}", bufs=3)
                            if ablate == "noreduce":
                                nc.vector.tensor_copy(out=ar[0:1, :], in_=gtw[0:1, :])
                            else:
                                nc.gpsimd.partition_all_reduce(
                                    ar[:], gtw[:], channels=128,
                                    reduce_op=bass_isa.ReduceOp.add)
                            parts.append(ar)
                        s_blk = p_sseg.tile([1, BLK], f32, tag="sblk")
                        nc.vector.tensor_tensor(out=s_blk[:], in0=parts[0][0:1, :],
                                                in1=parts[1][0:1, :], op=Alu.add)
                        nc.vector.tensor_tensor(out=s_blk[:], in0=s_blk[:],
                                                in1=parts[2][0:1, :], op=Alu.add)
                        nc.vector.tensor_tensor(out=s_blk[:], in0=s_blk[:],
                                                in1=parts[3][0:1, :], op=Alu.add)
                    flat, rem, off = t0, BLK, 0
                    while rem > 0:
                        g = flat // (128 * SEG_LEN)
                        p = (flat // SEG_LEN) % 128
                        c = flat % SEG_LEN
                        if c:
                            n = min(SEG_LEN - c, rem)
                            dst = s_pack[p:p + 1, g * SEG_LEN + c: g * SEG_LEN + c + n]
                        else:
                            rows = min(rem // SEG_LEN, 128 - p)
                            if rows:
                                n = rows * SEG_LEN
                                dst = s_pack[p:p + rows, g * SEG_LEN:(g + 1) * SEG_LEN]
                            else:
                                n = rem
                                dst = s_pack[p:p + 1, g * SEG_LEN: g * SEG_LEN + n]
                        nc.sync.dma_start(out=dst, in_=s_blk[0:1, off:off + n])
                        flat += n; rem -= n; off += n

                    while (units_done < n_units and
                           (units[units_done][0] + units[units_done][1]) * SEG_LEN
                           <= (blk + 1) * BLK):
                        pending += emit_topk(units_done)
                        units_done += 1
                    for _ in range(drain_rate):
                        if pending:
                            emit_gather(*pending.pop(0))

                # flush any pending topk/gather units emitted by the block loop
                while units_done < n_units:
                    pending += emit_topk(units_done)
                    units_done += 1
                while pending:
                    emit_gather(*pending.pop(0))


def _build_v2(nc, n_samples, reps=1, ablate=""):
    """v2: mm1 with x^T as stationary -> psum u in [token, k] layout.

    - pass 1: fp22 x fp22 (f32r) main product, 8 MMs per 128-token chunk.
    - pass 2+3 fused: one fp8e4m3 DoubleRow pass computes Wh*xl + Wl*xh
      into a separate psum at scale 2^18 (both corrections packed as the
      two DoubleRow sub-chunks). ~0.5 cyc/row instead of 2 full passes.
    - combine: gpsimd (pc*2^-18 + b1), DVE add into pu, ACT exact Gelu.
    - mm2: DVE tensor_tensor_reduce with w2 broadcast along free dim,
      accumulating scores into s_mat columns; PE-transposes 4 columns per
      block into score rows; DMA into s_pack [seg, 18] layout.
    - topk/gather: same DVE max8 machinery as v1.
    """
    import concourse.mybir as mybir
    from concourse.tile import TileContext
    from concourse.masks import make_identity
    import concourse.bass as bass

    f32 = mybir.dt.float32
    f32r = mybir.dt.float32r
    e4m3 = mybir.dt.float8e4
    u32 = mybir.dt.uint32
    i32 = mybir.dt.int32
    AF = mybir.ActivationFunctionType
    Alu = mybir.AluOpType
    DR = mybir.MatmulPerfMode.DoubleRow

    tokens = n_samples * N_TOK
    assert tokens % BLK == 0
    n_blocks = tokens // BLK
    n_segs = n_samples * 32
    assert n_segs % 128 == 0
    n_groups = n_segs // 128
    n_chunks = tokens // 128            # 128-token chunks (4 per block)
    SC = 2.0 ** 18                      # corr psum scale
    SW = 64.0                           # Wh8 = e4m3(W * SW)
    SX = SC / SW                        # xl8 = e4m3(xl * SX)

    x_ext = nc.declare_dram_parameter(
        "image_features", [n_samples, N_TOK, H_DIM], f32, isOutput=False)
    w1_ext = nc.declare_dram_parameter("W1", [K_DIM, H_DIM], f32, isOutput=False)
    b1_ext = nc.declare_dram_parameter("b1", [K_DIM], f32, isOutput=False)
    w2_ext = nc.declare_dram_parameter("W2", [1, K_DIM], f32, isOutput=False)
    out_ext = nc.declare_dram_parameter(
        "out", [n_samples, QUOTA * 32, H_DIM], f32, isOutput=True)

    x_flat = x_ext.ap().rearrange("b n h -> (b n) h")            # [tokens, H]
    out_sj = out_ext.ap().rearrange("b (sg j) h -> (b sg) j h", j=QUOTA)

    n_hc = H_DIM // 128   # 8 h chunks

    with TileContext(nc) as tc:
        with (
            tc.tile_pool(name="wprep", bufs=1) as wprep,
            tc.tile_pool(name="weights", bufs=1) as wpool,
            tc.tile_pool(name="xnat", bufs=8) as p_xnat,
            tc.tile_pool(name="xthi", bufs=16) as p_xthi,
            tc.tile_pool(name="drx", bufs=16) as p_drx,
            tc.tile_pool(name="gt", bufs=3) as p_gt,
            tc.tile_pool(name="gw", bufs=2) as p_gw,
            tc.tile_pool(name="smat", bufs=2) as p_smat,
            tc.tile_pool(name="srow", bufs=2) as p_srow,
            tc.tile_pool(name="ps_xt", bufs=2, space="PSUM") as ps_xt,
            tc.tile_pool(name="ps_u", bufs=2, space="PSUM") as ps_u,
            tc.tile_pool(name="ps_c", bufs=2, space="PSUM") as ps_c,
            tc.tile_pool(name="ps_st", bufs=2, space="PSUM") as ps_st,
            tc.tile_pool(name="topk", bufs=3) as p_topk,
            tc.tile_pool(name="got", bufs=4) as p_got,
        ):
            # ---- one-time weight prep ----
            ident = wprep.tile([128, 128], f32, tag="ident")
            make_identity(nc, ident)

            w1nat = [p_xnat.tile([128, H_DIM], f32, tag="xn", name=f"w1n{c}")
                     for c in range(4)]
            for c in range(4):
                nc.sync.dma_start(out=w1nat[c][:], in_=w1_ext[c * 128:(c + 1) * 128, :])
            # per hc: W1T chunk [128h, 512k]; hi (f32r) + fp8 corr pair
            w1t_hi = [wpool.tile([128, K_DIM], f32r, tag=f"w1th{h}", name=f"w1th{h}")
                      for h in range(n_hc)]
            w_corr = [wpool.tile([128, 2, K_DIM], e4m3, tag=f"wc{h}", name=f"wc{h}")
                      for h in range(n_hc)]
            for hc in range(n_hc):
                pw = ps_xt.tile([128, K_DIM], f32, tag="psxt", name=f"pw{hc}")
                for c in range(4):
                    nc.tensor.transpose(
                        out=pw[:, c * 128:(c + 1) * 128],
                        in_=w1nat[c][:, hc * 128:(hc + 1) * 128],
                        identity=ident[:])
                nc.scalar.copy(out=w1t_hi[hc][:], in_=pw[:])
                wsc = p_gw.tile([128, K_DIM], f32, tag="gw", name=f"wsc{hc}")
                nc.vector.tensor_scalar(wsc[:], pw[:], SW, scalar2=None, op0=Alu.mult)
                nc.scalar.activation(out=w_corr[hc][:, 0, :], in_=wsc[:],
                                     func=AF.Copy, scale=1.0)
                wl = p_gt.tile([128, K_DIM], f32, tag="gt", name=f"wl{hc}")
                nc.vector.tensor_tensor(out=wl[:], in0=pw[:],
                                        in1=w1t_hi[hc].bitcast(f32)[:],
                                        op=Alu.subtract)
                nc.vector.tensor_scalar(wl[:], wl[:], SC, scalar2=None, op0=Alu.mult)
                nc.scalar.activation(out=w_corr[hc][:, 1, :], in_=wl[:],
                                     func=AF.Copy, scale=1.0)

            # b1 / w2 broadcast tiles [128, 512]
            b1row = wpool.tile([1, K_DIM], f32, tag="b1row")
            w2row = wpool.tile([1, K_DIM], f32, tag="w2row")
            nc.sync.dma_start(out=b1row[:], in_=b1_ext[:].rearrange("(o k) -> o k", o=1))
            nc.sync.dma_start(out=w2row[:], in_=w2_ext[:])
            # broadcast rows to 128 partitions by log-doubling SBUF->SBUF DMAs
            b1b = wpool.tile([128, K_DIM], f32, tag="b1b")
            w2b = wpool.tile([128, K_DIM], f32, tag="w2b")
            nc.sync.dma_start(out=b1b[0:1, :], in_=b1row[:])
            nc.sync.dma_start(out=w2b[0:1, :], in_=w2row[:])
            p_done = 1
            while p_done < 128:
                n = min(p_done, 128 - p_done)
                nc.sync.dma_start(out=b1b[p_done:p_done + n, :], in_=b1b[0:n, :])
                nc.sync.dma_start(out=w2b[p_done:p_done + n, :], in_=w2b[0:n, :])
                p_done += n

            # seg base indices: base[p, g] = (g*128 + p) * 18
            base_i = wpool.tile([128, n_groups], i32, tag="basei")
            base_f = wpool.tile([128, n_groups], f32, tag="basef")
            nc.gpsimd.iota(base_i[:], pattern=[[128 * SEG_LEN, n_groups]],
                           base=0, channel_multiplier=SEG_LEN)
            nc.vector.tensor_copy(out=base_f[:], in_=base_i[:])

            s_pack = wpool.tile([128, n_groups * SEG_LEN], f32, tag="spack")

            for rep in range(reps):
                s_mat = p_smat.tile([128, n_chunks], f32, tag="smat",
                                    name=f"smat{rep}")
                SEGS_U = 64
                units = [(s, SEGS_U) for s in range(0, n_segs - SEGS_U, SEGS_U)]
                units += [(n_segs - SEGS_U, 32), (n_segs - 32, 32)]
                n_units = len(units)
                units_done = 0
                pending = []
                drain_rate = -(-n_units * QUOTA // n_blocks)

                def emit_topk(u):
                    seg0, nsg = units[u]
                    p0 = seg0 % 128
                    g = seg0 // 128
                    sseg = s_pack[p0:p0 + nsg, g * SEG_LEN:(g + 1) * SEG_LEN]
                    m8 = p_topk.tile([nsg, 8], f32, tag="m8", name=f"m8_{rep}_{u}")
                    i8 = p_topk.tile([nsg, 8], u32, tag="i8", name=f"i8_{rep}_{u}")
                    zap = p_topk.tile([nsg, SEG_LEN], f32, tag="zap", name=f"zap{rep}_{u}")
                    m8b = p_topk.tile([nsg, 8], f32, tag="m8b", name=f"m8b{rep}_{u}")
                    i8b = p_topk.tile([nsg, 8], u32, tag="i8b", name=f"i8b{rep}_{u}")
                    iv = p_topk.tile([nsg, QUOTA], f32, tag="iv", name=f"iv{rep}_{u}")
                    ivn = p_topk.tile([nsg, QUOTA], f32, tag="ivn", name=f"ivn{rep}_{u}")
                    asc = p_topk.tile([nsg, 8], f32, tag="asc", name=f"asc{rep}_{u}")
                    gidx_f = p_topk.tile([nsg, QUOTA], f32, tag="gidxf", name=f"gf{rep}_{u}")
                    gidx = p_topk.tile([nsg, QUOTA], u32, tag="gidx", name=f"gi{rep}_{u}",
                                       bufs=4)

                    nc.vector.max(out=m8[:], in_=sseg)
                    nc.vector.max_index(out=i8[:], in_max=m8[:], in_values=sseg)
                    nc.vector.match_replace(out=zap[:], in_to_replace=m8[:],
                                            in_values=sseg, imm_value=-1e30)
                    nc.vector.max(out=m8b[:], in_=zap[:])
                    nc.vector.max_index(out=i8b[:], in_max=m8b[:], in_values=zap[:])
                    nc.vector.tensor_copy(out=iv[:, 0:8], in_=i8[:])
                    nc.vector.tensor_copy(out=iv[:, 8:9], in_=i8b[:, 0:1])
                    nc.vector.tensor_scalar(ivn[:], iv[:], -1.0, scalar2=None,
                                            op0=Alu.mult)
                    nc.vector.max(out=asc[:], in_=ivn[:])
                    nc.vector.tensor_scalar(gidx_f[:, 0:8], asc[:], -1.0, scalar2=None,
                                            op0=Alu.mult)
                    nc.vector.reduce_max(out=gidx_f[:, 8:9], in_=iv[:],
                                         axis=mybir.AxisListType.X)
                    nc.vector.tensor_scalar(gidx_f[:], gidx_f[:],
                                            base_f[p0:p0 + nsg, g:g + 1],
                                            scalar2=None, op0=Alu.add)
                    nc.vector.tensor_copy(out=gidx[:], in_=gidx_f[:])
                    return [(u, gidx, j) for j in range(QUOTA)]

                def emit_gather(u, gidx, j):
                    if ablate == "nogather":
                        return
                    seg0, nsg = units[u]
                    got = p_got.tile([nsg, H_DIM], f32, tag="got", name=f"got{rep}_{u}_{j}")
                    nc.gpsimd.indirect_dma_start(
                        out=got[:], out_offset=None, in_=x_flat,
                        in_offset=bass.IndirectOffsetOnAxis(ap=gidx[:, j:j + 1], axis=0))
                    nc.sync.dma_start(
                        out=out_sj[seg0:seg0 + nsg, j, :], in_=got[:])

                for blk in range(n_blocks):
                    t0 = blk * BLK
                    xn = [p_xnat.tile([128, H_DIM], f32, tag="xn",
                                      name=f"xn{rep}_{blk}_{t}") for t in range(4)]
                    for t in range(4):
                        nc.sync.dma_start(
                            out=xn[t][:], in_=x_flat[t0 + t * 128: t0 + (t + 1) * 128, :])
                    xthi_l, drx_l = [], []
                    for hc in range(n_hc):
                        pxt = ps_xt.tile([128, BLK], f32, tag="psxt")
                        for t in range(4):
                            nc.tensor.transpose(
                                out=pxt[:, t * 128:(t + 1) * 128],
                                in_=xn[t][:, hc * 128:(hc + 1) * 128],
                                identity=ident[:])
                        # xthi_s = fp22(x * SX); pass-1 result is SX-scaled and
                        # rescaled in the combine step.
                        xthi = p_xthi.tile([128, BLK], f32r, tag="xthi")
                        nc.vector.tensor_scalar(xthi[:], pxt[:], SX, scalar2=None,
                                                op0=Alu.mult)
                        drx = p_drx.tile([128, 2, BLK], e4m3, tag="drx")
                        nc.vector.scalar_tensor_tensor(
                            out=drx[:, 0, :], in0=pxt[:], scalar=SX,
                            in1=xthi.bitcast(f32)[:], op0=Alu.mult,
                            op1=Alu.subtract)
                        nc.scalar.activation(out=drx[:, 1, :], in_=pxt[:],
                                             func=AF.Copy, scale=1.0)
                        xthi_l.append(xthi)
                        drx_l.append(drx)

                    for tcn in range(4):
                        ts_ = slice(tcn * 128, (tcn + 1) * 128)
                        pu = ps_u.tile([128, BLK], f32, tag="psu")
                        pc = ps_c.tile([128, BLK], f32, tag="psc")
                        for hc in range(n_hc):
                            nc.tensor.matmul(
                                out=pu[:], lhsT=xthi_l[hc][:, ts_],
                                rhs=w1t_hi[hc][:],
                                start=(hc == 0), stop=(hc == n_hc - 1))
                        for hc in range(n_hc):
                            nc.tensor.matmul(
                                out=pc[:], lhsT=drx_l[hc][:, :, ts_],
                                rhs=w_corr[hc][:], perf_mode=DR,
                                start=(hc == 0), stop=(hc == n_hc - 1))
                        uc = p_gw.tile([128, BLK], f32, tag="uc", name=f"uc{rep}_{blk}_{tcn}",
                                       bufs=3)
                        nc.vector.scalar_tensor_tensor(
                            out=uc[:], in0=pc[:], scalar=1.0 / SC, in1=b1b[:],
                            op0=Alu.mult, op1=Alu.add)
                        nc.vector.scalar_tensor_tensor(
                            out=uc[:], in0=pu[:], scalar=1.0 / SX, in1=uc[:],
                            op0=Alu.mult, op1=Alu.add)
                        gt = p_gt.tile([128, BLK], f32, tag="gt")
                        nc.scalar.activation(out=gt[:], in_=uc[:], func=AF.Gelu,
                                             bias=0.0, scale=1.0)
                        gw = p_gw.tile([128, BLK], f32, tag="gw")
                        nc.vector.tensor_tensor(out=gw[:], in0=gt[:], in1=w2b[:],
                                                op=Alu.mult)
                        ci = blk * 4 + tcn
                        nc.vector.reduce_sum(out=s_mat[:, ci:ci + 1], in_=gw[:],
                                             axis=mybir.AxisListType.X)

                    # transpose this block's 4 score columns -> [4, 128] rows
                    pst = ps_st.tile([4, 128], f32, tag="psst")
                    nc.tensor.transpose(out=pst[:], in_=s_mat[:, blk * 4:blk * 4 + 4],
                                        identity=ident[:])
                    srow = p_srow.tile([4, 128], f32, tag="srow")
                    nc.scalar.copy(out=srow[:], in_=pst[:])
                    for r in range(4):
                        flat, rem, off = t0 + r * 128, 128, 0
                        while rem > 0:
                            g = flat // (128 * SEG_LEN)
                            p = (flat // SEG_LEN) % 128
                            c = flat % SEG_LEN
                            if c:
                                n = min(SEG_LEN - c, rem)
                                dst = s_pack[p:p + 1, g * SEG_LEN + c: g * SEG_LEN + c + n]
                            else:
                                rows = min(rem // SEG_LEN, 128 - p)
                                if rows:
                                    n = rows * SEG_LEN
                                    dst = s_pack[p:p + rows, g * SEG_LEN:(g + 1) * SEG_LEN]
                                else:
                                    n = rem
                                    dst = s_pack[p:p + 1, g * SEG_LEN: g * SEG_LEN + n]
                            nc.sync.dma_start(out=dst, in_=srow[r:r + 1, off:off + n])
                            flat += n; rem -= n; off += n

                    while (units_done < n_units and
                           (units[units_done][0] + units[units_done][1]) * SEG_LEN
                           <= (blk + 1) * BLK):
                        pending += emit_topk(units_done)
                        units_done += 1
                    for _ in range(drain_rate):
                        if pending:
                            emit_gather(*pending.pop(0))

                while units_done < n_units:
                    pending += emit_topk(units_done)
                    units_done += 1
                while pending:
                    emit_gather(*pending.pop(0))


def _build_v3(nc, n_samples, sc_mode="f32r", scat="col", reps=1, ablate="",
              mm2_exact=True):
    """v3: seg-major blocks + SBUF->DRAM OOB-skip scatter (no HBM gather re-read).

    - x loaded per 128-segment group as two half tiles [128, 9*1024] f32
      (partition = segment, 9 tokens' features along free dim). One DMA each,
      72KB/partition-pair contiguous descriptors.
    - scoring: B-orientation v1-style. PE transposes x^T per 3-column span
      (384 tokens); psum -> SBUF cast to fp16 (RN, ~11-bit significand) or
      f32r (fp22 trunc). mm1: lhsT=W1T chunks (same dtype), rhs=x^T spans,
      psum [128k, 384tok]. ACT exact-Gelu w/ per-partition b1 bias. mm2 on
      PE: lhsT=w2 column, rhs=gelu-out, accumulated into s_ps[span, :].
    - score routing: s_ps -> SBUF; 3 PE transposes [6,128]->[128,6] land
      scores as s_pack[seg, i] columns (stride-3 slices).
    - top-9: DVE max8/match_replace/max8 -> threshold = 9th max; sel mask;
      tensor_tensor_scan prefix sum -> rank; dest row = seg*9 + rank, or
      2^21 (OOB) when unselected. Final f32->i32 cast on GPSIMD (descriptor
      generator only sees same-engine writes reliably).
    - output: indirect scatter SBUF->DRAM, bounds_check=rows-1,
      oob_is_err=False silently drops unselected rows. Write traffic =
      selected tokens only; x is never re-read from HBM.
    """
    import concourse.mybir as mybir
    from concourse.tile import TileContext
    from concourse.masks import make_identity
    import concourse.bass as bass
    import concourse.bass_isa as bass_isa

    f32 = mybir.dt.float32
    f32r = mybir.dt.float32r
    f16 = mybir.dt.float16
    i32 = mybir.dt.int32
    AF = mybir.ActivationFunctionType
    Alu = mybir.AluOpType

    n_segs = n_samples * 32
    assert n_segs % 128 == 0
    n_groups = n_segs // 128
    OUT_ROWS = n_segs * QUOTA
    BIG = float(1 << 21)
    SPW = 3 * 128  # span = 3 token-columns
    sdt = f16 if sc_mode == "fp16" else f32r
    n_hc = H_DIM // 128
    n_kc = K_DIM // 128

    x_ext = nc.declare_dram_parameter(
        "image_features", [n_samples, N_TOK, H_DIM], f32, isOutput=False)
    w1_ext = nc.declare_dram_parameter("W1", [K_DIM, H_DIM], f32, isOutput=False)
    b1_ext = nc.declare_dram_parameter("b1", [K_DIM], f32, isOutput=False)
    w2_ext = nc.declare_dram_parameter("W2", [1, K_DIM], f32, isOutput=False)
    out_ext = nc.declare_dram_parameter("out", [OUT_ROWS, H_DIM], f32, isOutput=True)
    x_flat = x_ext.ap().rearrange("b n h -> (b n) h")

    with TileContext(nc) as tc:
        with (
            tc.tile_pool(name="wprep", bufs=1) as wprep,
            tc.tile_pool(name="weights", bufs=1) as wpool,
            tc.tile_pool(name="xh", bufs=4) as p_xh,
            tc.tile_pool(name="xt", bufs=10) as p_xt,
            tc.tile_pool(name="usb", bufs=3) as p_usb,
            tc.tile_pool(name="gt", bufs=3) as p_gt,
            tc.tile_pool(name="gw", bufs=2) as p_gw,
            tc.tile_pool(name="spk", bufs=2) as p_spk,
            tc.tile_pool(name="tk", bufs=2) as p_tk,
            tc.tile_pool(name="ps_xt", bufs=2, space="PSUM") as ps_xt,
            tc.tile_pool(name="ps_u", bufs=3, space="PSUM") as ps_u,
        ):
            ident = wprep.tile([128, 128], f32, tag="ident")
            make_identity(nc, ident)

            # ---- one-time W1^T prep (exact f32 transposes, cast once) ----
            # w1nat borrows an xh ring slot (one-time use, recycled by loads)
            w1nat = p_xh.tile([128, 9 * H_DIM], f32, tag="xh", name="w1nat")
            for c in range(n_kc):
                nc.sync.dma_start(out=w1nat[:, c * H_DIM:(c + 1) * H_DIM],
                                  in_=w1_ext[c * 128:(c + 1) * 128, :])
            w1t = [wpool.tile([128, K_DIM], sdt, tag=f"w1t{h}", name=f"w1t{h}")
                   for h in range(n_hc)]
            for hc in range(n_hc):
                pw = ps_xt.tile([128, K_DIM], f32, tag="pw", name=f"pw{hc}",
                                bufs=1)
                for c in range(n_kc):
                    nc.tensor.transpose(
                        out=pw[:, c * 128:(c + 1) * 128],
                        in_=w1nat[:, c * H_DIM + hc * 128: c * H_DIM + hc * 128 + 128],
                        identity=ident[:])
                nc.scalar.copy(out=w1t[hc][:], in_=pw[:])

            # b1 / w2 broadcast to all partitions: [128, 512]
            b1b = wpool.tile([128, K_DIM], f32, tag="b1b")
            w2b = wpool.tile([128, K_DIM], f32, tag="w2b")
            nc.sync.dma_start(out=b1b[0:1, :],
                              in_=b1_ext[:].rearrange("(o k) -> o k", o=1))
            nc.sync.dma_start(out=w2b[0:1, :], in_=w2_ext[:])
            p_done = 1
            while p_done < 128:
                n = min(p_done, 128 - p_done)
                nc.sync.dma_start(out=b1b[p_done:p_done + n, :], in_=b1b[0:n, :])
                nc.sync.dma_start(out=w2b[p_done:p_done + n, :], in_=w2b[0:n, :])
                p_done += n

            # base_m[p, g] = (g*128 + p)*9 - 1 - BIG
            base_i = wpool.tile([128, n_groups], i32, tag="basei")
            base_f = wpool.tile([128, n_groups], f32, tag="basef")
            base_m = wpool.tile([128, n_groups], f32, tag="basem")
            nc.gpsimd.iota(base_i[:], pattern=[[128 * QUOTA, n_groups]],
                           base=0, channel_multiplier=QUOTA)
            nc.vector.tensor_copy(out=base_f[:], in_=base_i[:])
            nc.vector.tensor_scalar(base_m[:], base_f[:], -1.0 - BIG, scalar2=None,
                                    op0=Alu.add)
            zero18 = wpool.tile([128, SEG_LEN], f32, tag="z18")
            nc.vector.memset(zero18[:], 0.0)

            for rep in range(reps):
                for g in range(n_groups):
                    xh = [p_xh.tile([128, 9 * H_DIM], f32, tag="xh",
                                    name=f"xh{rep}_{g}_{h}") for h in range(2)]
                    src3 = x_flat[g * 2304:(g + 1) * 2304, :].rearrange(
                        "(p i) h -> p i h", i=SEG_LEN)
                    for h in range(2):
                        nc.sync.dma_start(
                            out=xh[h][:].rearrange("p (i h2) -> p i h2", h2=H_DIM),
                            in_=src3[:, h * 9:(h + 1) * 9, :])

                    spk = p_spk.tile([128, SEG_LEN], f32, tag="spk",
                                     name=f"spk{rep}_{g}")

                    for s in range(6):
                        cols = [3 * s, 3 * s + 1, 3 * s + 2]
                        xts = []
                        for hc in range(n_hc):
                            pxt = ps_xt.tile([128, SPW], f32, tag="psxt")
                            for jc, i in enumerate(cols):
                                src = xh[i // 9]
                                off = (i % 9) * H_DIM + hc * 128
                                nc.tensor.transpose(
                                    out=pxt[:, jc * 128:(jc + 1) * 128],
                                    in_=src[:, off:off + 128],
                                    identity=ident[:])
                            xt_t = p_xt.tile([128, SPW], sdt, tag="xt")
                            nc.scalar.copy(out=xt_t[:], in_=pxt[:])
                            xts.append(xt_t)
                        # A-orientation: psum u [128 tok, 512 k] per column;
                        # k-reduce on DVE (free dim) straight into spk column.
                        for jc, i in enumerate(cols):
                            pu = ps_u.tile([128, K_DIM], f32, tag="psu")
                            cs = slice(jc * 128, (jc + 1) * 128)
                            for hc in range(n_hc):
                                nc.tensor.matmul(
                                    out=pu[:], lhsT=xts[hc][:, cs],
                                    rhs=w1t[hc][:], start=(hc == 0),
                                    stop=(hc == n_hc - 1))
                            u_sb = p_usb.tile([128, K_DIM], f32, tag="usb")
                            nc.vector.scalar_tensor_tensor(
                                out=u_sb[:], in0=pu[:], scalar=1.0, in1=b1b[:],
                                op0=Alu.mult, op1=Alu.add)
                            gt = p_gt.tile([128, K_DIM], f32, tag="gt")
                            nc.scalar.activation(out=gt[:], in_=u_sb[:],
                                                 func=AF.Gelu, bias=0.0, scale=1.0)
                            gw = p_gw.tile([128, K_DIM], f32, tag="gw")
                            nc.vector.tensor_tensor(out=gw[:], in0=gt[:],
                                                    in1=w2b[:], op=Alu.mult)
                            nc.vector.reduce_sum(out=spk[:, i:i + 1], in_=gw[:],
                                                 axis=mybir.AxisListType.X)

                    # top-9 threshold + rank -> scatter dest rows
                    m8 = p_tk.tile([128, 8], f32, tag="m8")
                    zap = p_tk.tile([128, SEG_LEN], f32, tag="zap")
                    m9 = p_tk.tile([128, 8], f32, tag="m9")
                    sel = p_tk.tile([128, SEG_LEN], f32, tag="sel")
                    scn = p_tk.tile([128, SEG_LEN], f32, tag="scn")
                    d1 = p_tk.tile([128, SEG_LEN], f32, tag="d1")
                    destf = p_tk.tile([128, SEG_LEN], f32, tag="destf")
                    dest_i = p_tk.tile([128, SEG_LEN], i32, tag="desti", bufs=3)
                    nc.vector.max(out=m8[:], in_=spk[:])
                    nc.vector.match_replace(out=zap[:], in_to_replace=m8[:],
                                            in_values=spk[:], imm_value=-1e30)
                    nc.vector.max(out=m9[:], in_=zap[:])
                    nc.vector.tensor_scalar(sel[:], spk[:], m9[:, 0:1], scalar2=None,
                                            op0=Alu.is_ge)
                    nc.vector.tensor_tensor_scan(out=scn[:], data0=sel[:],
                                                 data1=zero18[:], initial=0.0,
                                                 op0=Alu.add, op1=Alu.add)
                    nc.vector.tensor_scalar(d1[:], scn[:], base_m[:, g:g + 1],
                                            scalar2=None, op0=Alu.add)
                    nc.vector.tensor_tensor(out=d1[:], in0=d1[:], in1=sel[:],
                                            op=Alu.mult)
                    nc.vector.tensor_scalar(destf[:], d1[:], BIG, scalar2=None,
                                            op0=Alu.add)
                    nc.gpsimd.tensor_copy(out=dest_i[:], in_=destf[:])

                    if ablate == "nogather":
                        continue
                    if scat == "batch":
                        for h in range(2):
                            nc.gpsimd.indirect_dma_start(
                                out=out_ext[:],
                                out_offset=bass.IndirectOffsetOnAxis(
                                    ap=dest_i[:, h * 9:(h + 1) * 9].rearrange(
                                        "p (i o) -> (p i) o", o=1), axis=0),
                                in_=xh[h][:].rearrange("p (i h2) -> (p i) h2",
                                                       h2=H_DIM),
                                in_offset=None,
                                bounds_check=OUT_ROWS - 1, oob_is_err=False)
                    else:
                        for i in range(SEG_LEN):
                            nc.gpsimd.indirect_dma_start(
                                out=out_ext[:],
                                out_offset=bass.IndirectOffsetOnAxis(
                                    ap=dest_i[:, i:i + 1], axis=0),
                                in_=xh[i // 9][:, (i % 9) * H_DIM:
                                               (i % 9 + 1) * H_DIM],
                                in_offset=None,
                                bounds_check=OUT_ROWS - 1, oob_is_err=False)


def _get_runner(n_samples, mm1_mode, mm2_mode, reps=1, ablate=""):
    key = (n_samples, mm1_mode, mm2_mode, reps, ablate)
    if key in _COMPILED:
        return _COMPILED[key]
    import concourse.bacc as bacc
    nc = bacc.Bacc()
    if mm1_mode.startswith("v3"):
        suf = mm1_mode[2:]
        sc_mode = "f32r" if "r" in suf else "fp16"
        scat = "batch" if "b" in suf else "col"
        if "n" in suf and not ablate:
            ablate = "nogather"
        _build_v3(nc, n_samples, sc_mode=sc_mode, scat=scat, reps=reps,
                  ablate=ablate, mm2_exact=("p" not in suf))
    elif mm1_mode == "v2":
        _build_v2(nc, n_samples, reps=reps, ablate=ablate)
    else:
        _build(nc, n_samples, mm1_mode, mm2_mode, reps=reps, ablate=ablate)
    nc.finalize()
    _COMPILED[key] = nc
    return nc


def kernel(image_features, W1, b1, W2, b2, target_num_tokens,
           mm1_mode="v2", mm2_mode="dve"):
    from concourse.bass_utils import run_bass_kernel_spmd

    x = np.ascontiguousarray(np.asarray(image_features, dtype=np.float32))
    W1 = np.ascontiguousarray(np.asarray(W1, dtype=np.float32))
    b1 = np.ascontiguousarray(np.asarray(b1, dtype=np.float32))
    W2 = np.ascontiguousarray(np.asarray(W2, dtype=np.float32))
    assert int(target_num_tokens) == QUOTA * 32
    Bt = x.shape[0]
    n_samples = Bt // N_CORES
    nc = _get_runner(n_samples, mm1_mode, mm2_mode)

    in_maps = []
    for c in range(N_CORES):
        in_maps.append({
            "image_features": x[c * n_samples:(c + 1) * n_samples],
            "W1": W1, "b1": b1, "W2": W2,
        })
    res = run_bass_kernel_spmd(nc, in_maps, core_ids=list(range(N_CORES)))
    outs = [np.asarray(res.results[c]["out"]).reshape(n_samples, QUOTA * 32, H_DIM)
            for c in range(N_CORES)]
    out = np.concatenate(outs, axis=0)
    return out.astype(image_features.dtype, copy=False)



# revision 24
# speedup vs baseline: 2.5123x; 2.5123x over previous
"""AdaptiveImageTokenPruner Trainium2 kernel (8 NeuronCores, data parallel).

Key facts used (verified against the reference):
- sigmoid, +b2, and per-sample min-max normalization are monotonic -> the
  top-k selection depends only on the pre-sigmoid logits s = W2 @ gelu(W1x+b1).
- the "adaptive tree" in the reference always splits to max depth -> fixed
  32 segments of 18 tokens per sample, top-9 per segment, indices ascending.
- per-segment top-9 via DVE max8/max_index/match_replace; token gather via
  indirect DMA from HBM.

Scorer matmul precision: PE is natively fp22 (11-bit mantissa inputs, exact
products, ~f32 accumulate). Plain float32 matmuls run at 4 cycles/row (exact);
float32r runs at 1 cycle/row but truncates inputs to 11 mantissa bits.
A hi/lo split (x = x_hi + x_lo, both fp22-exact) recovers near-f32 accuracy
in 2-3 passes at full rate. mm1_mode selects: "f32" | "f32r_x2" | "f32r_x3".
"""

import numpy as np

B_FULL, N_TOK, H_DIM = 256, 576, 1024
K_DIM = 512
SEG_LEN, QUOTA = 18, 9
N_CORES = 8
B_CORE = B_FULL // N_CORES          # 32 samples per core
BLK = 512                            # tokens per scoring block

_COMPILED = {}


def _build(nc, n_samples, mm1_mode="f32", mm2_mode="dve", reps=1, ablate=""):
    import concourse.mybir as mybir
    from concourse.tile import TileContext
    from concourse.masks import make_identity
    import concourse.bass as bass
    import concourse.bass_isa as bass_isa

    f32 = mybir.dt.float32
    f32r = mybir.dt.float32r
    f16 = mybir.dt.float16
    u32 = mybir.dt.uint32
    i32 = mybir.dt.int32
    AF = mybir.ActivationFunctionType
    Alu = mybir.AluOpType

    tokens = n_samples * N_TOK
    assert tokens % BLK == 0
    n_blocks = tokens // BLK
    n_segs = n_samples * 32
    assert n_segs % 128 == 0
    n_groups = n_segs // 128
    seg_per_group = 128

    x_ext = nc.declare_dram_parameter(
        "image_features", [n_samples, N_TOK, H_DIM], f32, isOutput=False)
    w1_ext = nc.declare_dram_parameter("W1", [K_DIM, H_DIM], f32, isOutput=False)
    b1_ext = nc.declare_dram_parameter("b1", [K_DIM], f32, isOutput=False)
    w2_ext = nc.declare_dram_parameter("W2", [1, K_DIM], f32, isOutput=False)
    out_ext = nc.declare_dram_parameter(
        "out", [n_samples, QUOTA * 32, H_DIM], f32, isOutput=True)

    x_flat = x_ext.ap().rearrange("b n h -> (b n) h")            # [tokens, H]
    out_sj = out_ext.ap().rearrange("b (sg j) h -> (b sg) j h", j=QUOTA)

    n_hc = H_DIM // 128   # 8 h chunks
    n_kc = K_DIM // 128   # 4 k chunks

    with TileContext(nc) as tc:
        with (
            tc.tile_pool(name="wprep", bufs=1) as wprep,
            tc.tile_pool(name="weights", bufs=1) as wpool,
            tc.tile_pool(name="xnat", bufs=7) as p_xnat,
            tc.tile_pool(name="xt", bufs=14) as p_xt,
            tc.tile_pool(name="xtlo", bufs=14) as p_xtlo,
            tc.tile_pool(name="gt", bufs=8) as p_gt,
            tc.tile_pool(name="ps_xt", bufs=5, space="PSUM") as ps_xt,
            tc.tile_pool(name="ps_u", bufs=3, space="PSUM") as ps_u,
            tc.tile_pool(name="sseg", bufs=3) as p_sseg,
            tc.tile_pool(name="topk", bufs=3) as p_topk,
            tc.tile_pool(name="got", bufs=4) as p_got,
        ):
            # ---- one-time weight prep ----
            ident = wprep.tile([128, 128], f32, tag="ident")
            make_identity(nc, ident)

            split = mm1_mode != "f32"
            sdt = f16 if mm1_mode.startswith("fp16") else f32r
            # W1T[hc] : [128h, 512k]  (hi/lo fp32r variants when splitting;
            # fp32r-typed writes round to fp22, exactly what the PE consumes)
            w1nat = [p_xnat.tile([128, H_DIM], f32, tag="xn", name=f"w1n{c}") for c in range(n_kc)]
            for c in range(n_kc):
                nc.sync.dma_start(out=w1nat[c][:], in_=w1_ext[c * 128:(c + 1) * 128, :])
            if split:
                w1t_hi = [wpool.tile([128, K_DIM], sdt, tag=f"w1th{h}", name=f"w1th{h}") for h in range(n_hc)]
                w1t_lo = [wpool.tile([128, K_DIM], sdt, tag=f"w1tl{h}", name=f"w1tl{h}") for h in range(n_hc)]
            else:
                w1t = [wpool.tile([128, K_DIM], f32, tag=f"w1t{h}", name=f"w1t{h}") for h in range(n_hc)]
            for hc in range(n_hc):
                pw = ps_xt.tile([128, K_DIM], f32, tag="psxt", name=f"pw{hc}")
                for c in range(n_kc):
                    nc.tensor.transpose(
                        out=pw[:, c * 128:(c + 1) * 128],
                        in_=w1nat[c][:, hc * 128:(hc + 1) * 128],
                        identity=ident[:])
                if split:
                    nc.vector.tensor_copy(out=w1t_hi[hc][:], in_=pw[:])
                    hi_rd = w1t_hi[hc].bitcast(f32) if sdt == f32r else w1t_hi[hc]
                    nc.vector.tensor_tensor(
                        out=w1t_lo[hc][:], in0=pw[:],
                        in1=hi_rd[:], op=Alu.subtract)
                else:
                    nc.scalar.copy(out=w1t[hc][:], in_=pw[:])

            # b1 -> [128, n_kc]; W2 -> [128, n_kc] (partition = k within chunk)
            b1_sb = wpool.tile([128, n_kc], f32, tag="b1")
            w2_sb = wpool.tile([128, n_kc], f32, tag="w2")
            for c in range(n_kc):
                nc.sync.dma_start(
                    out=b1_sb[:, c:c + 1],
                    in_=b1_ext[c * 128:(c + 1) * 128].rearrange("(p o) -> p o", o=1))
                nc.sync.dma_start(
                    out=w2_sb[:, c:c + 1],
                    in_=w2_ext[0:1, c * 128:(c + 1) * 128].rearrange("o (p q) -> (o p) q", q=1))

            # seg base indices: base[p, g] = (g*128 + p) * 18
            base_i = wpool.tile([128, n_groups], i32, tag="basei")
            base_f = wpool.tile([128, n_groups], f32, tag="basef")
            nc.gpsimd.iota(base_i[:], pattern=[[seg_per_group * SEG_LEN, n_groups]],
                           base=0, channel_multiplier=SEG_LEN)
            nc.vector.tensor_copy(out=base_f[:], in_=base_i[:])

            # scores packed [128 seg-partitions, n_groups*18]:
            # s_pack[p, g*18+i] = score of token (g*128+p)*18 + i
            s_pack = wpool.tile([128, n_groups * SEG_LEN], f32, tag="spack")

            for rep in range(reps):
                # ---- topk/gather machinery: 64-segment units, interleaved with
                # scoring so gather DMA traffic spreads instead of bursting ----
                SEGS_U = 64
                assert n_segs % SEGS_U == 0
                # unit list: 64-seg units, but the last 64 segs split into 2x32
                # so the pipeline tail (only the final unit's topk+gather runs
                # after the last score) is halved.
                units = [(s, SEGS_U) for s in range(0, n_segs - SEGS_U, SEGS_U)]
                units += [(n_segs - SEGS_U, 32), (n_segs - 32, 32)]
                n_units = len(units)
                units_done = 0
                pending = []
                drain_rate = -(-n_units * QUOTA // n_blocks)

                def emit_topk(u):
                    seg0, nsg = units[u]
                    p0 = seg0 % 128
                    g = seg0 // 128
                    sseg = s_pack[p0:p0 + nsg, g * SEG_LEN:(g + 1) * SEG_LEN]
                    m8 = p_topk.tile([nsg, 8], f32, tag="m8", name=f"m8_{rep}_{u}")
                    i8 = p_topk.tile([nsg, 8], u32, tag="i8", name=f"i8_{rep}_{u}")
                    zap = p_topk.tile([nsg, SEG_LEN], f32, tag="zap", name=f"zap{rep}_{u}")
                    m8b = p_topk.tile([nsg, 8], f32, tag="m8b", name=f"m8b{rep}_{u}")
                    i8b = p_topk.tile([nsg, 8], u32, tag="i8b", name=f"i8b{rep}_{u}")
                    iv = p_topk.tile([nsg, QUOTA], f32, tag="iv", name=f"iv{rep}_{u}")
                    ivn = p_topk.tile([nsg, QUOTA], f32, tag="ivn", name=f"ivn{rep}_{u}")
                    asc = p_topk.tile([nsg, 8], f32, tag="asc", name=f"asc{rep}_{u}")
                    gidx_f = p_topk.tile([nsg, QUOTA], f32, tag="gidxf", name=f"gf{rep}_{u}")
                    gidx = p_topk.tile([nsg, QUOTA], u32, tag="gidx", name=f"gi{rep}_{u}",
                                       bufs=4)

                    nc.vector.max(out=m8[:], in_=sseg)
                    nc.vector.max_index(out=i8[:], in_max=m8[:], in_values=sseg)
                    nc.vector.match_replace(out=zap[:], in_to_replace=m8[:],
                                            in_values=sseg, imm_value=-1e30)
                    nc.vector.max(out=m8b[:], in_=zap[:])
                    nc.vector.max_index(out=i8b[:], in_max=m8b[:], in_values=zap[:])
                    nc.vector.tensor_copy(out=iv[:, 0:8], in_=i8[:])
                    nc.vector.tensor_copy(out=iv[:, 8:9], in_=i8b[:, 0:1])
                    nc.vector.tensor_scalar(ivn[:], iv[:], -1.0, scalar2=None,
                                            op0=Alu.mult)
                    nc.vector.max(out=asc[:], in_=ivn[:])
                    nc.vector.tensor_scalar(gidx_f[:, 0:8], asc[:], -1.0, scalar2=None,
                                            op0=Alu.mult)
                    nc.vector.reduce_max(out=gidx_f[:, 8:9], in_=iv[:],
                                         axis=mybir.AxisListType.X)
                    nc.vector.tensor_scalar(gidx_f[:], gidx_f[:],
                                            base_f[p0:p0 + nsg, g:g + 1],
                                            scalar2=None, op0=Alu.add)
                    nc.vector.tensor_copy(out=gidx[:], in_=gidx_f[:])
                    return [(u, gidx, j) for j in range(QUOTA)]

                def emit_gather(u, gidx, j):
                    if ablate == "nogather":
                        return
                    seg0, nsg = units[u]
                    got = p_got.tile([nsg, H_DIM], f32, tag="got", name=f"got{rep}_{u}_{j}")
                    nc.gpsimd.indirect_dma_start(
                        out=got[:], out_offset=None, in_=x_flat,
                        in_offset=bass.IndirectOffsetOnAxis(ap=gidx[:, j:j + 1], axis=0))
                    nc.sync.dma_start(
                        out=out_sj[seg0:seg0 + nsg, j, :], in_=got[:])

                # ---- scoring over token blocks ----
                for blk in range(n_blocks):
                    t0 = blk * BLK
                    xn = [p_xnat.tile([128, H_DIM], f32, tag="xn", name=f"xn{rep}_{blk}_{t}") for t in range(4)]
                    for t in range(4):
                        nc.sync.dma_start(
                            out=xn[t][:], in_=x_flat[t0 + t * 128: t0 + (t + 1) * 128, :])
                    # transpose into [128h, 512tok] (+ hi/lo split)
                    xts, xts_lo = [], []
                    for hc in range(n_hc):
                        pxt = ps_xt.tile([128, BLK], f32, tag="psxt")
                        for t in range(4):
                            nc.tensor.transpose(
                                out=pxt[:, t * 128:(t + 1) * 128],
                                in_=xn[t][:, hc * 128:(hc + 1) * 128],
                                identity=ident[:])
                        if split:
                            xt_hi = p_xt.tile([128, BLK], sdt, tag="xt")
                            xt_lo = p_xtlo.tile([128, BLK], sdt, tag="xtlo")
                            nc.scalar.copy(out=xt_hi[:], in_=pxt[:])
                            hi_rd = xt_hi.bitcast(f32) if sdt == f32r else xt_hi
                            nc.vector.tensor_tensor(
                                out=xt_lo[:], in0=pxt[:], in1=hi_rd[:],
                                op=Alu.subtract)
                            xts.append(xt_hi)
                            xts_lo.append(xt_lo)
                        else:
                            xt = p_xt.tile([128, BLK], f32, tag="xt")
                            nc.scalar.copy(out=xt[:], in_=pxt[:])
                            xts.append(xt)

                    # u^T[kc] = sum_h W1T[h, kslice].T @ xT[h, :]  (+gelu -> g^T)
                    gts = []
                    for c in range(n_kc):
                        pu = ps_u.tile([128, BLK], f32, tag="psu")
                        ks = slice(c * 128, (c + 1) * 128)
                        if mm1_mode == "f32":
                            for hc in range(n_hc):
                                nc.tensor.matmul(
                                    out=pu[:], lhsT=w1t[hc][:, ks], rhs=xts[hc][:],
                                    start=(hc == 0), stop=(hc == n_hc - 1))
                        else:
                            passes = [(w1t_hi, xts)]
                            if mm1_mode not in ("f32r_x1",):
                                passes.append((w1t_hi, xts_lo))
                            if mm1_mode in ("f32r_x3", "fp16_x3"):
                                passes.append((w1t_lo, xts))
                            n_mm = len(passes) * n_hc
                            m = 0
                            for wv, xv in passes:
                                for hc in range(n_hc):
                                    nc.tensor.matmul(
                                        out=pu[:], lhsT=wv[hc][:, ks], rhs=xv[hc][:],
                                        start=(m == 0), stop=(m == n_mm - 1))
                                    m += 1
                        gt = p_gt.tile([128, BLK], f32, tag="gt")
                        nc.scalar.activation(out=gt[:], in_=pu[:], func=AF.Gelu,
                                             bias=b1_sb[:, c:c + 1], scale=1.0)
                        gts.append(gt)

                    # s[1, blk] = sum_k w2[k] * g^T[k, :]
                    if mm2_mode == "f32":
                        psc = ps_u.tile([1, BLK], f32, tag="pss")
                        for c in range(n_kc):
                            nc.tensor.matmul(
                                out=psc[:], lhsT=w2_sb[:, c:c + 1], rhs=gts[c][:],
                                start=(c == 0), stop=(c == n_kc - 1))
                        s_blk = p_sseg.tile([1, BLK], f32, tag="sblk")
                        nc.scalar.copy(out=s_blk[:], in_=psc[:])
                    else:
                        # w2-scale per partition (DVE), then cross-partition add
                        # (gpsimd axis=C), then combine the 4 k-chunk partials.
                        parts = []
                        for c in range(n_kc):
                            gtw = p_gt.tile([128, BLK], f32, tag="gtw", name=f"gtw{rep}_{blk}_{c}")
                            nc.vector.tensor_scalar(
                                gtw[:], gts[c][:], w2_sb[:, c:c + 1], scalar2=None,
                                op0=Alu.mult)
                            ar = p_gt.tile([128, BLK], f32, tag="arout",
                                           name=f"ar{rep}_{blk}_{c}", bufs=3)
                            if ablate == "noreduce":
                                nc.vector.tensor_copy(out=ar[0:1, :], in_=gtw[0:1, :])
                            else:
                                nc.gpsimd.partition_all_reduce(
                                    ar[:], gtw[:], channels=128,
                                    reduce_op=bass_isa.ReduceOp.add)
                            parts.append(ar)
                        s_blk = p_sseg.tile([1, BLK], f32, tag="sblk")
                        nc.vector.tensor_tensor(out=s_blk[:], in0=parts[0][0:1, :],
                                                in1=parts[1][0:1, :], op=Alu.add)
                        nc.vector.tensor_tensor(out=s_blk[:], in0=s_blk[:],
                                                in1=parts[2][0:1, :], op=Alu.add)
                        nc.vector.tensor_tensor(out=s_blk[:], in0=s_blk[:],
                                                in1=parts[3][0:1, :], op=Alu.add)
                    flat, rem, off = t0, BLK, 0
                    while rem > 0:
                        g = flat // (128 * SEG_LEN)
                        p = (flat // SEG_LEN) % 128
                        c = flat % SEG_LEN
                        if c:
                            n = min(SEG_LEN - c, rem)
                            dst = s_pack[p:p + 1, g * SEG_LEN + c: g * SEG_LEN + c + n]
                        else:
                            rows = min(rem // SEG_LEN, 128 - p)
                            if rows:
                                n = rows * SEG_LEN
                                dst = s_pack[p:p + rows, g * SEG_LEN:(g + 1) * SEG_LEN]
                            else:
                                n = rem
                                dst = s_pack[p:p + 1, g * SEG_LEN: g * SEG_LEN + n]
                        nc.sync.dma_start(out=dst, in_=s_blk[0:1, off:off + n])
                        flat += n; rem -= n; off += n

                    while (units_done < n_units and
                           (units[units_done][0] + units[units_done][1]) * SEG_LEN
                           <= (blk + 1) * BLK):
                        pending += emit_topk(units_done)
                        units_done += 1
                    for _ in range(drain_rate):
                        if pending:
                            emit_gather(*pending.pop(0))

                # flush any pending topk/gather units emitted by the block loop
                while units_done < n_units:
                    pending += emit_topk(units_done)
                    units_done += 1
                while pending:
                    emit_gather(*pending.pop(0))


def _build_v2(nc, n_samples, reps=1, ablate=""):
    """v2: mm1 with x^T as stationary -> psum u in [token, k] layout.

    - pass 1: fp22 x fp22 (f32r) main product, 8 MMs per 128-token chunk.
    - pass 2+3 fused: one fp8e4m3 DoubleRow pass computes Wh*xl + Wl*xh
      into a separate psum at scale 2^18 (both corrections packed as the
      two DoubleRow sub-chunks). ~0.5 cyc/row instead of 2 full passes.
    - combine: gpsimd (pc*2^-18 + b1), DVE add into pu, ACT exact Gelu.
    - mm2: DVE tensor_tensor_reduce with w2 broadcast along free dim,
      accumulating scores into s_mat columns; PE-transposes 4 columns per
      block into score rows; DMA into s_pack [seg, 18] layout.
    - topk/gather: same DVE max8 machinery as v1.
    """
    import concourse.mybir as mybir
    from concourse.tile import TileContext
    from concourse.masks import make_identity
    import concourse.bass as bass

    f32 = mybir.dt.float32
    f32r = mybir.dt.float32r
    e4m3 = mybir.dt.float8e4
    u32 = mybir.dt.uint32
    i32 = mybir.dt.int32
    AF = mybir.ActivationFunctionType
    Alu = mybir.AluOpType
    DR = mybir.MatmulPerfMode.DoubleRow

    tokens = n_samples * N_TOK
    assert tokens % BLK == 0
    n_blocks = tokens // BLK
    n_segs = n_samples * 32
    assert n_segs % 128 == 0
    n_groups = n_segs // 128
    n_chunks = tokens // 128            # 128-token chunks (4 per block)
    SC = 2.0 ** 18                      # corr psum scale
    SW = 64.0                           # Wh8 = e4m3(W * SW)
    SX = SC / SW                        # xl8 = e4m3(xl * SX)

    x_ext = nc.declare_dram_parameter(
        "image_features", [n_samples, N_TOK, H_DIM], f32, isOutput=False)
    w1_ext = nc.declare_dram_parameter("W1", [K_DIM, H_DIM], f32, isOutput=False)
    b1_ext = nc.declare_dram_parameter("b1", [K_DIM], f32, isOutput=False)
    w2_ext = nc.declare_dram_parameter("W2", [1, K_DIM], f32, isOutput=False)
    out_ext = nc.declare_dram_parameter(
        "out", [n_samples, QUOTA * 32, H_DIM], f32, isOutput=True)

    x_flat = x_ext.ap().rearrange("b n h -> (b n) h")            # [tokens, H]
    out_sj = out_ext.ap().rearrange("b (sg j) h -> (b sg) j h", j=QUOTA)

    n_hc = H_DIM // 128   # 8 h chunks

    with TileContext(nc) as tc:
        with (
            tc.tile_pool(name="wprep", bufs=1) as wprep,
            tc.tile_pool(name="weights", bufs=1) as wpool,
            tc.tile_pool(name="xnat", bufs=8) as p_xnat,
            tc.tile_pool(name="xthi", bufs=16) as p_xthi,
            tc.tile_pool(name="drx", bufs=16) as p_drx,
            tc.tile_pool(name="gt", bufs=3) as p_gt,
            tc.tile_pool(name="gw", bufs=2) as p_gw,
            tc.tile_pool(name="smat", bufs=2) as p_smat,
            tc.tile_pool(name="srow", bufs=2) as p_srow,
            tc.tile_pool(name="ps_xt", bufs=2, space="PSUM") as ps_xt,
            tc.tile_pool(name="ps_u", bufs=2, space="PSUM") as ps_u,
            tc.tile_pool(name="ps_c", bufs=2, space="PSUM") as ps_c,
            tc.tile_pool(name="ps_st", bufs=2, space="PSUM") as ps_st,
            tc.tile_pool(name="topk", bufs=3) as p_topk,
            tc.tile_pool(name="got", bufs=4) as p_got,
        ):
            # ---- one-time weight prep ----
            ident = wprep.tile([128, 128], f32, tag="ident")
            make_identity(nc, ident)

            w1nat = [p_xnat.tile([128, H_DIM], f32, tag="xn", name=f"w1n{c}")
                     for c in range(4)]
            for c in range(4):
                nc.sync.dma_start(out=w1nat[c][:], in_=w1_ext[c * 128:(c + 1) * 128, :])
            # per hc: W1T chunk [128h, 512k]; hi (f32r) + fp8 corr pair
            w1t_hi = [wpool.tile([128, K_DIM], f32r, tag=f"w1th{h}", name=f"w1th{h}")
                      for h in range(n_hc)]
            w_corr = [wpool.tile([128, 2, K_DIM], e4m3, tag=f"wc{h}", name=f"wc{h}")
                      for h in range(n_hc)]
            for hc in range(n_hc):
                pw = ps_xt.tile([128, K_DIM], f32, tag="psxt", name=f"pw{hc}")
                for c in range(4):
                    nc.tensor.transpose(
                        out=pw[:, c * 128:(c + 1) * 128],
                        in_=w1nat[c][:, hc * 128:(hc + 1) * 128],
                        identity=ident[:])
                nc.scalar.copy(out=w1t_hi[hc][:], in_=pw[:])
                wsc = p_gw.tile([128, K_DIM], f32, tag="gw", name=f"wsc{hc}")
                nc.vector.tensor_scalar(wsc[:], pw[:], SW, scalar2=None, op0=Alu.mult)
                nc.scalar.activation(out=w_corr[hc][:, 0, :], in_=wsc[:],
                                     func=AF.Copy, scale=1.0)
                wl = p_gt.tile([128, K_DIM], f32, tag="gt", name=f"wl{hc}")
                nc.vector.tensor_tensor(out=wl[:], in0=pw[:],
                                        in1=w1t_hi[hc].bitcast(f32)[:],
                                        op=Alu.subtract)
                nc.vector.tensor_scalar(wl[:], wl[:], SC, scalar2=None, op0=Alu.mult)
                nc.scalar.activation(out=w_corr[hc][:, 1, :], in_=wl[:],
                                     func=AF.Copy, scale=1.0)

            # b1 / w2 broadcast tiles [128, 512]
            b1row = wpool.tile([1, K_DIM], f32, tag="b1row")
            w2row = wpool.tile([1, K_DIM], f32, tag="w2row")
            nc.sync.dma_start(out=b1row[:], in_=b1_ext[:].rearrange("(o k) -> o k", o=1))
            nc.sync.dma_start(out=w2row[:], in_=w2_ext[:])
            # broadcast rows to 128 partitions by log-doubling SBUF->SBUF DMAs
            b1b = wpool.tile([128, K_DIM], f32, tag="b1b")
            w2b = wpool.tile([128, K_DIM], f32, tag="w2b")
            nc.sync.dma_start(out=b1b[0:1, :], in_=b1row[:])
            nc.sync.dma_start(out=w2b[0:1, :], in_=w2row[:])
            p_done = 1
            while p_done < 128:
                n = min(p_done, 128 - p_done)
                nc.sync.dma_start(out=b1b[p_done:p_done + n, :], in_=b1b[0:n, :])
                nc.sync.dma_start(out=w2b[p_done:p_done + n, :], in_=w2b[0:n, :])
                p_done += n

            # seg base indices: base[p, g] = (g*128 + p) * 18
            base_i = wpool.tile([128, n_groups], i32, tag="basei")
            base_f = wpool.tile([128, n_groups], f32, tag="basef")
            nc.gpsimd.iota(base_i[:], pattern=[[128 * SEG_LEN, n_groups]],
                           base=0, channel_multiplier=SEG_LEN)
            nc.vector.tensor_copy(out=base_f[:], in_=base_i[:])

            s_pack = wpool.tile([128, n_groups * SEG_LEN], f32, tag="spack")

            for rep in range(reps):
                s_mat = p_smat.tile([128, n_chunks], f32, tag="smat",
                                    name=f"smat{rep}")
                SEGS_U = 64
                units = [(s, SEGS_U) for s in range(0, n_segs - SEGS_U, SEGS_U)]
                units += [(n_segs - SEGS_U, 32), (n_segs - 32, 32)]
                n_units = len(units)
                units_done = 0
                pending = []
                drain_rate = -(-n_units * QUOTA // n_blocks)

                def emit_topk(u):
                    seg0, nsg = units[u]
                    p0 = seg0 % 128
                    g = seg0 // 128
                    sseg = s_pack[p0:p0 + nsg, g * SEG_LEN:(g + 1) * SEG_LEN]
                    m8 = p_topk.tile([nsg, 8], f32, tag="m8", name=f"m8_{rep}_{u}")
                    i8 = p_topk.tile([nsg, 8], u32, tag="i8", name=f"i8_{rep}_{u}")
                    zap = p_topk.tile([nsg, SEG_LEN], f32, tag="zap", name=f"zap{rep}_{u}")
                    m8b = p_topk.tile([nsg, 8], f32, tag="m8b", name=f"m8b{rep}_{u}")
                    i8b = p_topk.tile([nsg, 8], u32, tag="i8b", name=f"i8b{rep}_{u}")
                    iv = p_topk.tile([nsg, QUOTA], f32, tag="iv", name=f"iv{rep}_{u}")
                    ivn = p_topk.tile([nsg, QUOTA], f32, tag="ivn", name=f"ivn{rep}_{u}")
                    asc = p_topk.tile([nsg, 8], f32, tag="asc", name=f"asc{rep}_{u}")
                    gidx_f = p_topk.tile([nsg, QUOTA], f32, tag="gidxf", name=f"gf{rep}_{u}")
                    gidx = p_topk.tile([nsg, QUOTA], u32, tag="gidx", name=f"gi{rep}_{u}",
                                       bufs=4)

                    nc.vector.max(out=m8[:], in_=sseg)
                    nc.vector.max_index(out=i8[:], in_max=m8[:], in_values=sseg)
                    nc.vector.match_replace(out=zap[:], in_to_replace=m8[:],
                                            in_values=sseg, imm_value=-1e30)
                    nc.vector.max(out=m8b[:], in_=zap[:])
                    nc.vector.max_index(out=i8b[:], in_max=m8b[:], in_values=zap[:])
                    nc.vector.tensor_copy(out=iv[:, 0:8], in_=i8[:])
                    nc.vector.tensor_copy(out=iv[:, 8:9], in_=i8b[:, 0:1])
                    nc.vector.tensor_scalar(ivn[:], iv[:], -1.0, scalar2=None,
                                            op0=Alu.mult)
                    nc.vector.max(out=asc[:], in_=ivn[:])
                    nc.vector.tensor_scalar(gidx_f[:, 0:8], asc[:], -1.0, scalar2=None,
                                            op0=Alu.mult)
                    nc.vector.reduce_max(out=gidx_f[:, 8:9], in_=iv[:],
                                         axis=mybir.AxisListType.X)
                    nc.vector.tensor_scalar(gidx_f[:], gidx_f[:],
                                            base_f[p0:p0 + nsg, g:g + 1],
                                            scalar2=None, op0=Alu.add)
                    nc.vector.tensor_copy(out=gidx[:], in_=gidx_f[:])
                    return [(u, gidx, j) for j in range(QUOTA)]

                def emit_gather(u, gidx, j):
                    if ablate == "nogather":
                        return
                    seg0, nsg = units[u]
                    got = p_got.tile([nsg, H_DIM], f32, tag="got", name=f"got{rep}_{u}_{j}")
                    nc.gpsimd.indirect_dma_start(
                        out=got[:], out_offset=None, in_=x_flat,
                        in_offset=bass.IndirectOffsetOnAxis(ap=gidx[:, j:j + 1], axis=0))
                    nc.sync.dma_start(
                        out=out_sj[seg0:seg0 + nsg, j, :], in_=got[:])

                for blk in range(n_blocks):
                    t0 = blk * BLK
                    xn = [p_xnat.tile([128, H_DIM], f32, tag="xn",
                                      name=f"xn{rep}_{blk}_{t}") for t in range(4)]
                    for t in range(4):
                        nc.sync.dma_start(
                            out=xn[t][:], in_=x_flat[t0 + t * 128: t0 + (t + 1) * 128, :])
                    xthi_l, drx_l = [], []
                    for hc in range(n_hc):
                        pxt = ps_xt.tile([128, BLK], f32, tag="psxt")
                        for t in range(4):
                            nc.tensor.transpose(
                                out=pxt[:, t * 128:(t + 1) * 128],
                                in_=xn[t][:, hc * 128:(hc + 1) * 128],
                                identity=ident[:])
                        # xthi_s = fp22(x * SX); pass-1 result is SX-scaled and
                        # rescaled in the combine step.
                        xthi = p_xthi.tile([128, BLK], f32r, tag="xthi")
                        nc.vector.tensor_scalar(xthi[:], pxt[:], SX, scalar2=None,
                                                op0=Alu.mult)
                        drx = p_drx.tile([128, 2, BLK], e4m3, tag="drx")
                        nc.vector.scalar_tensor_tensor(
                            out=drx[:, 0, :], in0=pxt[:], scalar=SX,
                            in1=xthi.bitcast(f32)[:], op0=Alu.mult,
                            op1=Alu.subtract)
                        nc.scalar.activation(out=drx[:, 1, :], in_=pxt[:],
                                             func=AF.Copy, scale=1.0)
                        xthi_l.append(xthi)
                        drx_l.append(drx)

                    for tcn in range(4):
                        ts_ = slice(tcn * 128, (tcn + 1) * 128)
                        pu = ps_u.tile([128, BLK], f32, tag="psu")
                        pc = ps_c.tile([128, BLK], f32, tag="psc")
                        for hc in range(n_hc):
                            nc.tensor.matmul(
                                out=pu[:], lhsT=xthi_l[hc][:, ts_],
                                rhs=w1t_hi[hc][:],
                                start=(hc == 0), stop=(hc == n_hc - 1))
                        for hc in range(n_hc):
                            nc.tensor.matmul(
                                out=pc[:], lhsT=drx_l[hc][:, :, ts_],
                                rhs=w_corr[hc][:], perf_mode=DR,
                                start=(hc == 0), stop=(hc == n_hc - 1))
                        uc = p_gw.tile([128, BLK], f32, tag="uc", name=f"uc{rep}_{blk}_{tcn}",
                                       bufs=3)
                        nc.vector.scalar_tensor_tensor(
                            out=uc[:], in0=pc[:], scalar=1.0 / SC, in1=b1b[:],
                            op0=Alu.mult, op1=Alu.add)
                        nc.vector.scalar_tensor_tensor(
                            out=uc[:], in0=pu[:], scalar=1.0 / SX, in1=uc[:],
                            op0=Alu.mult, op1=Alu.add)
                        gt = p_gt.tile([128, BLK], f32, tag="gt")
                        nc.scalar.activation(out=gt[:], in_=uc[:], func=AF.Gelu,
                                             bias=0.0, scale=1.0)
                        gw = p_gw.tile([128, BLK], f32, tag="gw")
                        nc.vector.tensor_tensor(out=gw[:], in0=gt[:], in1=w2b[:],
                                                op=Alu.mult)
                        ci = blk * 4 + tcn
                        nc.vector.reduce_sum(out=s_mat[:, ci:ci + 1], in_=gw[:],
                                             axis=mybir.AxisListType.X)

                    # transpose this block's 4 score columns -> [4, 128] rows
                    pst = ps_st.tile([4, 128], f32, tag="psst")
                    nc.tensor.transpose(out=pst[:], in_=s_mat[:, blk * 4:blk * 4 + 4],
                                        identity=ident[:])
                    srow = p_srow.tile([4, 128], f32, tag="srow")
                    nc.scalar.copy(out=srow[:], in_=pst[:])
                    for r in range(4):
                        flat, rem, off = t0 + r * 128, 128, 0
                        while rem > 0:
                            g = flat // (128 * SEG_LEN)
                            p = (flat // SEG_LEN) % 128
                            c = flat % SEG_LEN
                            if c:
                                n = min(SEG_LEN - c, rem)
                                dst = s_pack[p:p + 1, g * SEG_LEN + c: g * SEG_LEN + c + n]
                            else:
                                rows = min(rem // SEG_LEN, 128 - p)
                                if rows:
                                    n = rows * SEG_LEN
                                    dst = s_pack[p:p + rows, g * SEG_LEN:(g + 1) * SEG_LEN]
                                else:
                                    n = rem
                                    dst = s_pack[p:p + 1, g * SEG_LEN: g * SEG_LEN + n]
                            nc.sync.dma_start(out=dst, in_=srow[r:r + 1, off:off + n])
                            flat += n; rem -= n; off += n

                    while (units_done < n_units and
                           (units[units_done][0] + units[units_done][1]) * SEG_LEN
                           <= (blk + 1) * BLK):
                        pending += emit_topk(units_done)
                        units_done += 1
                    for _ in range(drain_rate):
                        if pending:
                            emit_gather(*pending.pop(0))

                while units_done < n_units:
                    pending += emit_topk(units_done)
                    units_done += 1
                while pending:
                    emit_gather(*pending.pop(0))


def _build_v3(nc, n_samples, sc_mode="f32r", scat="col", reps=1, ablate="",
              mm2_exact=True):
    """v3: seg-major blocks + SBUF->DRAM OOB-skip scatter (no HBM gather re-read).

    - x loaded per 128-segment group as two half tiles [128, 9*1024] f32
      (partition = segment, 9 tokens' features along free dim). One DMA each,
      72KB/partition-pair contiguous descriptors.
    - scoring: B-orientation v1-style. PE transposes x^T per 3-column span
      (384 tokens); psum -> SBUF cast to fp16 (RN, ~11-bit significand) or
      f32r (fp22 trunc). mm1: lhsT=W1T chunks (same dtype), rhs=x^T spans,
      psum [128k, 384tok]. ACT exact-Gelu w/ per-partition b1 bias. mm2 on
      PE: lhsT=w2 column, rhs=gelu-out, accumulated into s_ps[span, :].
    - score routing: s_ps -> SBUF; 3 PE transposes [6,128]->[128,6] land
      scores as s_pack[seg, i] columns (stride-3 slices).
    - top-9: DVE max8/match_replace/max8 -> threshold = 9th max; sel mask;
      tensor_tensor_scan prefix sum -> rank; dest row = seg*9 + rank, or
      2^21 (OOB) when unselected. Final f32->i32 cast on GPSIMD (descriptor
      generator only sees same-engine writes reliably).
    - output: indirect scatter SBUF->DRAM, bounds_check=rows-1,
      oob_is_err=False silently drops unselected rows. Write traffic =
      selected tokens only; x is never re-read from HBM.
    """
    import concourse.mybir as mybir
    from concourse.tile import TileContext
    from concourse.masks import make_identity
    import concourse.bass as bass
    import concourse.bass_isa as bass_isa

    f32 = mybir.dt.float32
    f32r = mybir.dt.float32r
    f16 = mybir.dt.float16
    i32 = mybir.dt.int32
    AF = mybir.ActivationFunctionType
    Alu = mybir.AluOpType

    n_segs = n_samples * 32
    assert n_segs % 128 == 0
    n_groups = n_segs // 128
    OUT_ROWS = n_segs * QUOTA
    BIG = float(1 << 21)
    SPW = 3 * 128  # span = 3 token-columns
    sdt = f16 if sc_mode == "fp16" else f32r
    n_hc = H_DIM // 128
    n_kc = K_DIM // 128

    x_ext = nc.declare_dram_parameter(
        "image_features", [n_samples, N_TOK, H_DIM], f32, isOutput=False)
    w1_ext = nc.declare_dram_parameter("W1", [K_DIM, H_DIM], f32, isOutput=False)
    b1_ext = nc.declare_dram_parameter("b1", [K_DIM], f32, isOutput=False)
    w2_ext = nc.declare_dram_parameter("W2", [1, K_DIM], f32, isOutput=False)
    out_ext = nc.declare_dram_parameter("out", [OUT_ROWS, H_DIM], f32, isOutput=True)
    x_flat = x_ext.ap().rearrange("b n h -> (b n) h")

    with TileContext(nc) as tc:
        with (
            tc.tile_pool(name="wprep", bufs=1) as wprep,
            tc.tile_pool(name="weights", bufs=1) as wpool,
            tc.tile_pool(name="xh", bufs=4) as p_xh,
            tc.tile_pool(name="xt", bufs=10) as p_xt,
            tc.tile_pool(name="usb", bufs=3) as p_usb,
            tc.tile_pool(name="gt", bufs=3) as p_gt,
            tc.tile_pool(name="gw", bufs=2) as p_gw,
            tc.tile_pool(name="spk", bufs=2) as p_spk,
            tc.tile_pool(name="tk", bufs=2) as p_tk,
            tc.tile_pool(name="ps_xt", bufs=2, space="PSUM") as ps_xt,
            tc.tile_pool(name="ps_u", bufs=3, space="PSUM") as ps_u,
        ):
            ident = wprep.tile([128, 128], f32, tag="ident")
            make_identity(nc, ident)

            # ---- one-time W1^T prep (exact f32 transposes, cast once) ----
            # w1nat borrows an xh ring slot (one-time use, recycled by loads)
            w1nat = p_xh.tile([128, 9 * H_DIM], f32, tag="xh", name="w1nat")
            for c in range(n_kc):
                nc.sync.dma_start(out=w1nat[:, c * H_DIM:(c + 1) * H_DIM],
                                  in_=w1_ext[c * 128:(c + 1) * 128, :])
            w1t = [wpool.tile([128, K_DIM], sdt, tag=f"w1t{h}", name=f"w1t{h}")
                   for h in range(n_hc)]
            for hc in range(n_hc):
                pw = ps_xt.tile([128, K_DIM], f32, tag="pw", name=f"pw{hc}",
                                bufs=1)
                for c in range(n_kc):
                    nc.tensor.transpose(
                        out=pw[:, c * 128:(c + 1) * 128],
                        in_=w1nat[:, c * H_DIM + hc * 128: c * H_DIM + hc * 128 + 128],
                        identity=ident[:])
                nc.scalar.copy(out=w1t[hc][:], in_=pw[:])

            # b1 / w2 broadcast to all partitions: [128, 512]
            b1b = wpool.tile([128, K_DIM], f32, tag="b1b")
            w2b = wpool.tile([128, K_DIM], f32, tag="w2b")
            nc.sync.dma_start(out=b1b[0:1, :],
                              in_=b1_ext[:].rearrange("(o k) -> o k", o=1))
            nc.sync.dma_start(out=w2b[0:1, :], in_=w2_ext[:])
            p_done = 1
            while p_done < 128:
                n = min(p_done, 128 - p_done)
                nc.sync.dma_start(out=b1b[p_done:p_done + n, :], in_=b1b[0:n, :])
                nc.sync.dma_start(out=w2b[p_done:p_done + n, :], in_=w2b[0:n, :])
                p_done += n

            # base_m[p, g] = (g*128 + p)*9 - 1 - BIG
            base_i = wpool.tile([128, n_groups], i32, tag="basei")
            base_f = wpool.tile([128, n_groups], f32, tag="basef")
            base_m = wpool.tile([128, n_groups], f32, tag="basem")
            nc.gpsimd.iota(base_i[:], pattern=[[128 * QUOTA, n_groups]],
                           base=0, channel_multiplier=QUOTA)
            nc.vector.tensor_copy(out=base_f[:], in_=base_i[:])
            nc.vector.tensor_scalar(base_m[:], base_f[:], -1.0 - BIG, scalar2=None,
                                    op0=Alu.add)
            zero18 = wpool.tile([128, SEG_LEN], f32, tag="z18")
            nc.vector.memset(zero18[:], 0.0)
            colidx_i = wpool.tile([128, SEG_LEN], i32, tag="cixi")
            colidx = wpool.tile([128, SEG_LEN], f32, tag="cix")
            nc.gpsimd.iota(colidx_i[:], pattern=[[1, SEG_LEN]], base=0,
                           channel_multiplier=0)
            nc.vector.tensor_copy(out=colidx[:], in_=colidx_i[:])

            for rep in range(reps):
                for g in range(n_groups):
                    xh = [p_xh.tile([128, 9 * H_DIM], f32, tag="xh",
                                    name=f"xh{rep}_{g}_{h}") for h in range(2)]
                    src3 = x_flat[g * 2304:(g + 1) * 2304, :].rearrange(
                        "(p i) h -> p i h", i=SEG_LEN)
                    for h in range(2):
                        nc.sync.dma_start(
                            out=xh[h][:].rearrange("p (i h2) -> p i h2", h2=H_DIM),
                            in_=src3[:, h * 9:(h + 1) * 9, :])

                    spk = p_spk.tile([128, SEG_LEN], f32, tag="spk",
                                     name=f"spk{rep}_{g}")

                    n_spans = 0 if ablate == "fixsel" else 6
                    for s in range(n_spans):
                        cols = [3 * s, 3 * s + 1, 3 * s + 2]
                        xts = []
                        for hc in range(n_hc):
                            pxt = ps_xt.tile([128, SPW], f32, tag="psxt")
                            for jc, i in enumerate(cols):
                                src = xh[i // 9]
                                off = (i % 9) * H_DIM + hc * 128
                                nc.tensor.transpose(
                                    out=pxt[:, jc * 128:(jc + 1) * 128],
                                    in_=src[:, off:off + 128],
                                    identity=ident[:])
                            xt_t = p_xt.tile([128, SPW], sdt, tag="xt")
                            nc.scalar.copy(out=xt_t[:], in_=pxt[:])
                            xts.append(xt_t)
                        # A-orientation: psum u [128 tok, 512 k] per column;
                        # k-reduce on DVE (free dim) straight into spk column.
                        for jc, i in enumerate(cols):
                            pu = ps_u.tile([128, K_DIM], f32, tag="psu")
                            cs = slice(jc * 128, (jc + 1) * 128)
                            for hc in range(n_hc):
                                nc.tensor.matmul(
                                    out=pu[:], lhsT=xts[hc][:, cs],
                                    rhs=w1t[hc][:], start=(hc == 0),
                                    stop=(hc == n_hc - 1))
                            u_sb = p_usb.tile([128, K_DIM], f32, tag="usb")
                            nc.vector.scalar_tensor_tensor(
                                out=u_sb[:], in0=pu[:], scalar=1.0, in1=b1b[:],
                                op0=Alu.mult, op1=Alu.add)
                            gt = p_gt.tile([128, K_DIM], f32, tag="gt")
                            nc.scalar.activation(out=gt[:], in_=u_sb[:],
                                                 func=AF.Gelu, bias=0.0, scale=1.0)
                            gw = p_gw.tile([128, K_DIM], f32, tag="gw")
                            nc.vector.tensor_tensor(out=gw[:], in0=gt[:],
                                                    in1=w2b[:], op=Alu.mult)
                            nc.vector.reduce_sum(out=spk[:, i:i + 1], in_=gw[:],
                                                 axis=mybir.AxisListType.X)

                    # top-9 threshold + rank -> scatter dest rows
                    m8 = p_tk.tile([128, 8], f32, tag="m8")
                    zap = p_tk.tile([128, SEG_LEN], f32, tag="zap")
                    m9 = p_tk.tile([128, 8], f32, tag="m9")
                    sel = p_tk.tile([128, SEG_LEN], f32, tag="sel")
                    scn = p_tk.tile([128, SEG_LEN], f32, tag="scn")
                    d1 = p_tk.tile([128, SEG_LEN], f32, tag="d1")
                    destf = p_tk.tile([128, SEG_LEN], f32, tag="destf")
                    dest_i = p_tk.tile([128, SEG_LEN], i32, tag="desti", bufs=3)
                    if ablate == "fixsel":
                        nc.vector.tensor_scalar(sel[:], colidx[:], 8.5,
                                                scalar2=None, op0=Alu.is_lt)
                        nc.vector.tensor_scalar(scn[:], colidx[:], 1.0,
                                                scalar2=None, op0=Alu.add)
                    else:
                        nc.vector.max(out=m8[:], in_=spk[:])
                        nc.vector.match_replace(out=zap[:], in_to_replace=m8[:],
                                                in_values=spk[:], imm_value=-1e30)
                        nc.vector.max(out=m9[:], in_=zap[:])
                        nc.vector.tensor_scalar(sel[:], spk[:], m9[:, 0:1],
                                                scalar2=None, op0=Alu.is_ge)
                        nc.vector.tensor_tensor_scan(out=scn[:], data0=sel[:],
                                                     data1=zero18[:], initial=0.0,
                                                     op0=Alu.add, op1=Alu.add)
                    nc.vector.tensor_scalar(d1[:], scn[:], base_m[:, g:g + 1],
                                            scalar2=None, op0=Alu.add)
                    nc.vector.tensor_tensor(out=d1[:], in0=d1[:], in1=sel[:],
                                            op=Alu.mult)
                    nc.vector.tensor_scalar(destf[:], d1[:], BIG, scalar2=None,
                                            op0=Alu.add)
                    nc.gpsimd.tensor_copy(out=dest_i[:], in_=destf[:])

                    if ablate == "nogather":
                        continue
                    if scat == "batch":
                        for h in range(2):
                            nc.gpsimd.indirect_dma_start(
                                out=out_ext[:],
                                out_offset=bass.IndirectOffsetOnAxis(
                                    ap=dest_i[:, h * 9:(h + 1) * 9].rearrange(
                                        "p (i o) -> (p i) o", o=1), axis=0),
                                in_=xh[h][:].rearrange("p (i h2) -> (p i) h2",
                                                       h2=H_DIM),
                                in_offset=None,
                                bounds_check=OUT_ROWS - 1, oob_is_err=False)
                    else:
                        for i in range(SEG_LEN):
                            nc.gpsimd.indirect_dma_start(
                                out=out_ext[:],
                                out_offset=bass.IndirectOffsetOnAxis(
                                    ap=dest_i[:, i:i + 1], axis=0),
                                in_=xh[i // 9][:, (i % 9) * H_DIM:
                                               (i % 9 + 1) * H_DIM],
                                in_offset=None,
                                bounds_check=OUT_ROWS - 1, oob_is_err=False)


def _get_runner(n_samples, mm1_mode, mm2_mode, reps=1, ablate=""):
    key = (n_samples, mm1_mode, mm2_mode, reps, ablate)
    if key in _COMPILED:
        return _COMPILED[key]
    import concourse.bacc as bacc
    nc = bacc.Bacc()
    if mm1_mode.startswith("v3"):
        suf = mm1_mode[2:]
        sc_mode = "f32r" if "r" in suf else "fp16"
        scat = "batch" if "b" in suf else "col"
        if "n" in suf and not ablate:
            ablate = "nogather"
        if "f" in suf and not ablate:
            ablate = "fixsel"
        _build_v3(nc, n_samples, sc_mode=sc_mode, scat=scat, reps=reps,
                  ablate=ablate, mm2_exact=("p" not in suf))
    elif mm1_mode == "v2":
        _build_v2(nc, n_samples, reps=reps, ablate=ablate)
    else:
        _build(nc, n_samples, mm1_mode, mm2_mode, reps=reps, ablate=ablate)
    nc.finalize()
    _COMPILED[key] = nc
    return nc


def kernel(image_features, W1, b1, W2, b2, target_num_tokens,
           mm1_mode="v2", mm2_mode="dve"):
    from concourse.bass_utils import run_bass_kernel_spmd

    x = np.ascontiguousarray(np.asarray(image_features, dtype=np.float32))
    W1 = np.ascontiguousarray(np.asarray(W1, dtype=np.float32))
    b1 = np.ascontiguousarray(np.asarray(b1, dtype=np.float32))
    W2 = np.ascontiguousarray(np.asarray(W2, dtype=np.float32))
    assert int(target_num_tokens) == QUOTA * 32
    Bt = x.shape[0]
    n_samples = Bt // N_CORES
    nc = _get_runner(n_samples, mm1_mode, mm2_mode)

    in_maps = []
    for c in range(N_CORES):
        in_maps.append({
            "image_features": x[c * n_samples:(c + 1) * n_samples],
            "W1": W1, "b1": b1, "W2": W2,
        })
    res = run_bass_kernel_spmd(nc, in_maps, core_ids=list(range(N_CORES)))
    outs = [np.asarray(res.results[c]["out"]).reshape(n_samples, QUOTA * 32, H_DIM)
            for c in range(N_CORES)]
    out = np.concatenate(outs, axis=0)
    return out.astype(image_features.dtype, copy=False)

